# revision 19
# baseline (speedup 1.0000x reference)
"""GCN layer on 8 trn2 NeuronCores.

out = tanh( (D^-1/2 (adj+I) D^-1/2) @ H @ W.T + b ), N=8192, nin=nout=512.

v4 design:
- bf16 matmuls: 1 PE cycle/row instead of fp32's 4, and half the HBM
  traffic for the 256MB adjacency stream (the dominant input).
- The full symmetric normalization d[:,None]*(adj+I)*d[None,:] and the
  self-loop are folded into the host-side shard prep (one vectorized
  pass over adjT while casting to bf16): the device does no scaling or
  self-add at all.
- Stationary-operand swap: stage-1 matmul uses lhsT=H chunks (stationary)
  and rhs=adjacency strips (moving), producing Hm^T = (Ahat_rows @ H)^T
  directly in PSUM — no PE transposes anywhere.
- Stage 2 computes out^T = tanh(W @ Hm^T + b) with lhsT=W^T chunks; in
  this transposed layout the bias is per-partition and fuses into the
  tanh activation. Host transposes the per-core out^T blocks back.
- DMA: host packs every stream into [128 partitions, ...] contiguous
  layout so each transfer is 128 straight runs; 8 k-tiles ride one DMA
  (vs 205 small DMAs), issued from the HWDGE queues (SP: adjacency,
  Activation: H + weights) so no engine serializes on descriptor gen.
  The first k-tile groups are split 2/2/4 so the PE starts ~4us in
  instead of waiting for a full 8-tile transfer chain.

Per core: 8192-deep contraction, 2 r-tiles of 512 rows, 4 feature
chunks -> 4 PSUM accumulator banks + 2 stage-2 banks. PE ~ 278k cycles
~ 116us at 2.4GHz; DMA ~ 27MB ~ 82us at 332GB/s -> PE-bound.
"""

import sys

sys.path.insert(0, "/opt/trn_rl_repo")

import numpy as np
import ml_dtypes

from concourse import bass, bacc, tile, mybir
from concourse.bass_utils import run_bass_kernel_spmd

N = 8192
NIN = 512
NOUT = 512
NC = 8
RB = N // NC  # 1024 rows per core
RT = 2  # r-tiles per core (512 rows each)
RW = RB // RT  # 512 r-tile width (stage-1 moving dim)
KT = N // 128  # 64 k-tiles (contraction)
G = 8  # k-tiles per full DMA group
FC = NIN // 128  # 4 feature chunks
OC = NOUT // 128  # 4 output chunks
F32 = mybir.dt.float32
BF16 = mybir.dt.bfloat16
BF = ml_dtypes.bfloat16

# (start_kt, n_kt) DMA pieces: small leading pieces cut pipeline fill time.
RAMP_PLAN = [(0, 1), (1, 1), (2, 2), (4, 4)] + [(8 + G * i, G) for i in range(7)]
FULL_PLAN = [(G * i, G) for i in range(KT // G)]
# H stream pieces: 4-ktile chunks interleave more smoothly with the
# adjacency strips on the shared DMA engines than 8-ktile lumps.
HB_PLAN = [(0, 1), (1, 1), (2, 2), (4, 4)] + [(8 + 4 * i, 4) for i in range(14)]

_CACHED_NC = None


def _build(repeat=1):
    nc = bacc.Bacc(None, target_bir_lowering=False)

    # Packed per-core inputs (see _prep for layouts):
    # S[p, rt, kt, c]  = Ahat^T[kt*128+p, rt*512+c] (normalized, self-loops in)
    # Hb[p, kt, c]     = H[kt*128+p, c]
    # WT[p, f, o]      = W.T[f*128+p, o]
    # Bt[p, o]         = b[o*128+p]
    # OutT[p, o, rt, c] = out^T[o*128+p, rt*512+c]
    S = nc.dram_tensor("S", [128, RT, KT, RW], BF16, kind="ExternalInput")
    Hb = nc.dram_tensor("Hb", [128, KT, NIN], BF16, kind="ExternalInput")
    WT = nc.dram_tensor("WT", [128, FC, NOUT], BF16, kind="ExternalInput")
    Bt = nc.dram_tensor("Bt", [128, OC], F32, kind="ExternalInput")
    OutT = nc.dram_tensor("outT", [128, OC, RT, RW], F32, kind="ExternalOutput")

    with tile.TileContext(nc) as tc:
        with (
            tc.tile_pool(name="persist", bufs=1) as persist,
            tc.tile_pool(name="strip", bufs=4) as striper,
            tc.tile_pool(name="hm", bufs=2) as hmpool,
            tc.tile_pool(name="osb", bufs=4) as opool,
            tc.tile_pool(name="acc", bufs=1, space=bass.MemorySpace.PSUM) as pacc,
            tc.tile_pool(name="pout", bufs=2, space=bass.MemorySpace.PSUM) as pout,
        ):
            import contextlib

            rep_ctx = tc.For_i(0, repeat) if repeat > 1 else contextlib.nullcontext()
            with rep_ctx:
                # H pieces (Activation queue). wt/bt slot in after the three
                # small ramp pieces: needed only at the first r-tile boundary.
                hb = {}  # kt -> (tile, col offset)
                for pi, (k0, n) in enumerate(HB_PLAN):
                    t = persist.tile([128, n * NIN], BF16, name=f"hb{pi}")
                    nc.scalar.dma_start(t[:], Hb[:, k0 : k0 + n, :])
                    for j in range(n):
                        hb[k0 + j] = (t, j * NIN)
                    if pi == 2:
                        wt = persist.tile([128, FC * NOUT], BF16)
                        nc.scalar.dma_start(wt[:], WT[:, :, :])
                        bt = persist.tile([128, OC], F32)
                        nc.scalar.dma_start(bt[:], Bt[:, :])

                # Copy completion order (alternating Act / DVE engines) — the
                # next r-tile's first matmuls reuse the acc banks in this same
                # order so the handoff bubble is one semaphore hop, and the
                # final k-tile emits f-chunks in this order too so each
                # chunk's copy fires as early as possible.
                FORD = [0, 2, 1, 3]

                def stage1(rt, accs):
                    plan = RAMP_PLAN if rt == 0 else FULL_PLAN

                    def mm(kt, f, strip, j):
                        ht, off = hb[kt]
                        nc.tensor.matmul(
                            accs[f][:],
                            ht[:, off + f * 128 : off + (f + 1) * 128],
                            strip[:, j * RW : (j + 1) * RW],
                            start=(kt == 0),
                            stop=(kt == KT - 1),
                        )

                    for k0, n in plan:
                        strip = striper.tile([128, G * RW], BF16, name="strip")
                        nc.sync.dma_start(
                            strip[:, : n * RW], S[:, rt, k0 : k0 + n, :]
                        )
                        if k0 + n == KT:
                            # Last piece: group the final two k-tiles per
                            # f-chunk so each accumulation group closes (and
                            # its PSUM->SBUF copy starts) as early as
                            # possible before the r-tile boundary.
                            for j in range(n - 2):
                                for f in FORD:
                                    mm(k0 + j, f, strip, j)
                            for f in FORD:
                                for j in (n - 2, n - 1):
                                    mm(k0 + j, f, strip, j)
                        else:
                            for j in range(n):
                                for f in FORD:
                                    mm(k0 + j, f, strip, j)
                        yield

                def copies(accs):
                    # Drain PSUM accs to SBUF bf16 the moment each f-chunk's
                    # accumulation group closes; two engines in parallel.
                    hm = [None] * FC
                    for f in FORD:
                        hm_f = hmpool.tile([128, RW], BF16, name=f"hm{f}")
                        if f in (0, 1):
                            nc.scalar.copy(hm_f[:], accs[f][:])
                        else:
                            nc.vector.tensor_scalar_add(hm_f[:], accs[f][:], 0.0)
                        hm[f] = hm_f
                    return hm

                def stage2(rt, hm):
                    # Last r-tile: out-DMA configs go to the SP/Pool queues
                    # (idle by then) so they don't delay the tail tanhs on
                    # the Activation queue.
                    dma_eng = (
                        [nc.scalar, nc.sync, nc.gpsimd, nc.sync]
                        if rt == RT - 1
                        else [nc.scalar] * OC
                    )
                    for o in range(OC):
                        po = pout.tile([128, RW], F32)
                        for f in FORD:
                            nc.tensor.matmul(
                                po[:],
                                wt[:, f * NOUT + o * 128 : f * NOUT + (o + 1) * 128],
                                hm[f][:],
                                start=(f == FORD[0]),
                                stop=(f == FORD[-1]),
                            )
                        osb = opool.tile([128, RW], F32)
                        nc.scalar.activation(
                            osb[:],
                            po[:],
                            mybir.ActivationFunctionType.Tanh,
                            bias=bt[:, o : o + 1],
                        )
                        dma_eng[o].dma_start(OutT[:, o, rt, :], osb[:])

                # Software pipeline: r-tile 0's stage 2 is emitted two DMA
                # groups into r-tile 1's matmul stream, so the PE queue never
                # waits on the PSUM->SBUF copies mid-run; only the last
                # r-tile's epilogue is exposed as tail.
                accs0 = [pacc.tile([128, RW], F32, name=f"acc{f}") for f in range(FC)]
                for _ in stage1(0, accs0):
                    pass
                hm0 = copies(accs0)
                accs1 = [pacc.tile([128, RW], F32, name=f"acc{f}") for f in range(FC)]
                for pi, _ in enumerate(stage1(1, accs1)):
                    if pi == 1:
                        stage2(0, hm0)
                hm1 = copies(accs1)
                stage2(1, hm1)

    nc.compile()
    return nc


def _prep(H, adj_matrix, W, b):
    """Host shard prep: normalization, bf16 casts, packed per-core layouts."""
    H = np.ascontiguousarray(np.asarray(H, dtype=np.float32))
    adj = np.asarray(adj_matrix, dtype=np.float32)
    W = np.asarray(W, dtype=np.float32)
    b = np.asarray(b, dtype=np.float32)

    deg = adj.sum(axis=0, dtype=np.float32) + 1.0  # +1 self loop
    d = deg.astype(np.float32) ** -0.5
    d = np.where(np.isinf(d), np.float32(0.0), d).astype(np.float32)

    # Ahat^T = d * (adj^T + I) * d; scale+cast in one pass, then add the
    # scaled self-loops on the diagonal.
    ST = (adj.T * d[:, None] * d[None, :]).astype(BF)
    idx = np.arange(N)
    ST[idx, idx] += (d * d).astype(BF)

    # Pack: Hb[p, kt, c]; per-core S[p, rt, kt, c].
    Hbp = np.ascontiguousarray(H.astype(BF).reshape(KT, 128, NIN).transpose(1, 0, 2))
    WTp = np.ascontiguousarray(
        W.T.astype(BF).reshape(FC, 128, NOUT).transpose(1, 0, 2)
    )
    Btp = np.ascontiguousarray(b.reshape(OC, 128).T.astype(np.float32))

    in_maps = []
    for c in range(NC):
        r0, r1 = c * RB, (c + 1) * RB
        Sc = ST[:, r0:r1].reshape(KT, 128, RT, RW).transpose(1, 2, 0, 3)
        in_maps.append(
            {
                "S": np.ascontiguousarray(Sc),
                "Hb": Hbp,
                "WT": WTp,
                "Bt": Btp,
            }
        )
    return in_maps


def kernel(H, adj_matrix, W, b):
    global _CACHED_NC
    in_maps = _prep(H, adj_matrix, W, b)
    if _CACHED_NC is None:
        _CACHED_NC = _build()
    globals()["_LAST_IN_MAPS"] = in_maps
    res = run_bass_kernel_spmd(_CACHED_NC, in_maps, core_ids=list(range(NC)))
    # OutT[p, o, rt, c] -> out[rows_c, feat]: feat = o*128+p, row = rt*512+c.
    blocks = []
    for c in range(NC):
        ot = res.results[c]["outT"]  # [128, OC, RT, RW]
        blocks.append(ot.transpose(1, 0, 2, 3).reshape(NOUT, RB).T)
    return np.ascontiguousarray(np.concatenate(blocks, axis=0))
